# revision 1
# baseline (speedup 1.0000x reference)
"""Expert-parallel Gemma MoE kernel for 8 Trainium2 NeuronCores.

Strategy: one expert per core with capacity factor 1.0. Host gathers each
expert's routed tokens up to a fixed capacity (the mean expert load); the
few overflow tokens above capacity (~2%) are computed host-side in fp32
during the scatter-add (exact, standard MoE capacity truncation except
nothing is dropped). Each core runs the gated MLP (gate_up matmul ->
gelu_tanh * up -> down matmul) on its token slab in bf16 with fp32 PSUM
accumulation; host scatter-adds the weighted per-expert outputs back into
the full [T, H] fp32 output.

All matmuls keep tokens on the moving (free) dimension so no on-device
transposes are needed: the host supplies X^T, W_gu^T and W_d^T and the
device produces y^T.

Schedule notes (from perfetto traces):
- The PE runs gap-free at ~204ns per [128x128]x[128,480] bf16 matmul once
  fed, so the wins are at the edges: the head (the opening m-tiles are
  DMA-paced: ~1.5MB must land before m0 runs unstalled) and the tail.
- DMA completions are aggregate-paced across the HWDGE ring (fine-grained
  issue slicing cannot beat the cumulative drain), so loads are whole
  blocks in consumption order with x^T first.
- WARM_HEAD warm-up matmuls on a memset scratch tile hold the PE busy
  until the opening DMAs land: they ramp the p-state (0.65->2.4GHz needs
  ~3us of continuous busy; any >~1us idle resets it) so the real stream
  starts at full clock with zero gaps.
- Tail: the last down-proj row tile is computed in two column chunks so
  the final PSUM->SBUF copy + store DMA covers only 128 columns.
"""

import functools

import numpy as np
import ml_dtypes

from concourse import bacc, bass, tile
from concourse import mybir

# Problem constants (nn_Gemma4TextExperts: Gemma-style MoE).
T = 2048      # tokens
H = 1024      # hidden
I = 2048      # intermediate
E = 8         # experts = cores
TOPK = 2

P = 128       # SBUF partitions
NMAX = 512    # max moving free dim per matmul (one PSUM bank of fp32)
WARM_HEAD = 20    # warm-up matmuls: hold the PE busy (and its p-state
                  # ramp alive) until the opening DMAs land (~4.5us)

BF16 = mybir.dt.bfloat16
F32 = mybir.dt.float32

KH = H // P       # 8  k-tiles for the H contraction
KI = I // P       # 16 k-tiles for the I contraction
MGU = I // P      # 16 gate (and 16 up) output row tiles
MH = H // P       # 8  output row tiles of down

# gate/up column blocks: m0, m1-3, then 3x4-tile blocks (finer early
# blocks keep the first supply waits small).
GU_BLOCKS = [(0, 1), (1, 4), (4, 8), (8, 12), (12, 16)]


def _build_bass(cap: int):
    """Build the single-core Bass program for a cap-token slab (cap<=512)."""
    assert cap <= NMAX
    # Bacc (not raw Bass): its compile() runs generate_event_semaphores,
    # which splits multi-sem sync waits that TRN2 instructions can't carry.
    nc = bacc.Bacc()

    def wparam(name, nk, m0, m1):
        return nc.declare_dram_parameter(
            name, [P, nk, (m1 - m0) * P], BF16, isOutput=False)

    xt_d = nc.declare_dram_parameter("xt", [P, KH, cap], BF16, isOutput=False)
    wg_d = [wparam(f"wg{i}", KH, m0, m1) for i, (m0, m1) in enumerate(GU_BLOCKS)]
    wu_d = [wparam(f"wu{i}", KH, m0, m1) for i, (m0, m1) in enumerate(GU_BLOCKS)]
    wd_d = [nc.declare_dram_parameter(f"wd{j}", [P, KI, NMAX], BF16,
                                      isOutput=False) for j in range(2)]
    yt_d = nc.declare_dram_parameter("yt", [MH, P, cap], F32, isOutput=True)

    with tile.TileContext(nc) as tc:
        with (
            tc.tile_pool(name="wpool", bufs=1) as wpool,
            tc.tile_pool(name="xpool", bufs=1) as xpool,
            tc.tile_pool(name="hpool", bufs=1) as hpool,
            tc.tile_pool(name="gpool", bufs=16) as gpool,
            tc.tile_pool(name="upool", bufs=16) as upool,
            tc.tile_pool(name="opool", bufs=8) as opool,
            tc.tile_pool(name="ppool", bufs=2, space=bass.MemorySpace.PSUM) as ppool,
            tc.tile_pool(name="pwpool", bufs=1, space=bass.MemorySpace.PSUM) as pwpool,
        ):
            wg_sb = [
                wpool.tile([P, KH, (m1 - m0) * P], BF16, tag=f"wg{i}", name=f"wg{i}")
                for i, (m0, m1) in enumerate(GU_BLOCKS)
            ]
            wu_sb = [
                wpool.tile([P, KH, (m1 - m0) * P], BF16, tag=f"wu{i}", name=f"wu{i}")
                for i, (m0, m1) in enumerate(GU_BLOCKS)
            ]
            wd_sb = [
                wpool.tile([P, KI, NMAX], BF16, tag=f"wd{j}", name=f"wd{j}")
                for j in range(2)
            ]
            xt_sb = xpool.tile([P, KH, cap], BF16, tag="xt", name="xt")

            # Warm-up matmuls on a memset scratch tile keep the PE busy
            # (and its p-state ramp alive) until the opening DMAs land.
            # DMA completions are aggregate-paced across the ring, so the
            # opening needs ~1.5MB in flight before m0 can run unstalled.
            scratch = xpool.tile([P, NMAX], BF16, tag="warm", name="warm")
            nc.vector.memset(scratch[:, :], 0)
            pwarm = pwpool.tile([P, NMAX], F32, tag="pwarm")

            def warm(n):
                for _ in range(n):
                    nc.tensor.matmul(
                        pwarm[:, :], scratch[:, 0:P], scratch[:, :],
                        start=True, stop=True, skip_group_check=True,
                    )

            warm(WARM_HEAD)

            # Input DMAs, all on the SP HWDGE ring (stores use the ACT
            # ring; measured: routing loads to the ACT ring drains LATER).
            # Completions are aggregate-paced across the ring, so fine
            # slicing can't beat the cumulative drain: issue whole blocks
            # in consumption order with x^T (reused by every m-tile) first.
            nc.sync.dma_start(out=xt_sb[:, :, :], in_=xt_d[:, :, :])
            for i in range(0, len(GU_BLOCKS)):
                nc.sync.dma_start(out=wg_sb[i][:, :, :], in_=wg_d[i][:, :, :])
                nc.sync.dma_start(out=wu_sb[i][:, :, :], in_=wu_d[i][:, :, :])
            for j in range(2):
                nc.sync.dma_start(out=wd_sb[j][:, :, :], in_=wd_d[j][:, :, :])

            def gu_slice(sb_list, m):
                for i, (m0, m1) in enumerate(GU_BLOCKS):
                    if m0 <= m < m1:
                        return sb_list[i], (m - m0) * P
                raise AssertionError(m)

            # h^T tiles: [P, KI, cap] bf16 (the gelu(gate)*up result).
            h_sb = hpool.tile([P, KI, cap], BF16, tag="h")

            for m in range(MGU):  # 16 (gate, up) row-tile pairs
                gsb, go = gu_slice(wg_sb, m)
                usb, uo = gu_slice(wu_sb, m)
                pg = ppool.tile([P, cap], F32, tag="pg")
                pu = ppool.tile([P, cap], F32, tag="pu")
                for k in range(KH):
                    nc.tensor.matmul(
                        pg[:, :], gsb[:, k, go:go + P], xt_sb[:, k, :],
                        start=(k == 0), stop=(k == KH - 1),
                    )
                for k in range(KH):
                    nc.tensor.matmul(
                        pu[:, :], usb[:, k, uo:uo + P], xt_sb[:, k, :],
                        start=(k == 0), stop=(k == KH - 1),
                    )
                g_sb = gpool.tile([P, cap], BF16, tag="g")
                nc.scalar.activation(
                    g_sb[:, :], pg[:, :],
                    mybir.ActivationFunctionType.Gelu_apprx_tanh,
                )
                nc.vector.tensor_mul(h_sb[:, m, :], g_sb[:, :], pu[:, :])

            tailc = min(P, cap)
            for mh in range(MH):  # 8 output row tiles
                jd, od = mh // 4, (mh % 4) * P
                # The last tile is computed in two column chunks so the
                # final PSUM->SBUF copy + store DMA covers only 128 columns.
                cols = ([(0, cap - tailc), (cap - tailc, cap)]
                        if mh == MH - 1 and cap > tailc else [(0, cap)])
                for c0, c1 in cols:
                    py = ppool.tile([P, c1 - c0], F32, tag="py")
                    for k in range(KI):
                        nc.tensor.matmul(
                            py[:, :], wd_sb[jd][:, k, od:od + P],
                            h_sb[:, k, c0:c1],
                            start=(k == 0), stop=(k == KI - 1),
                        )
                    o_sb = opool.tile([P, c1 - c0], F32, tag="o")
                    nc.vector.tensor_copy(o_sb[:, :], py[:, :])
                    nc.scalar.dma_start(out=yt_d[mh, :, c0:c1], in_=o_sb[:, :])

    nc.finalize()
    return nc


@functools.lru_cache(maxsize=4)
def _get_program(cap: int):
    return _build_bass(cap)


def _capacity(tok_lists):
    total = sum(len(tl) for tl in tok_lists)
    cap = (total + len(tok_lists) - 1) // len(tok_lists)  # mean load (CF=1.0)
    cap = (cap + 15) // 16 * 16
    return min(cap, NMAX)


def prepare_in_maps(x, gup, dp, tok_lists, cap):
    """Per-core input dicts in the partition-major block layouts the
    device program expects (see _build_bass)."""
    in_maps = []
    for c in range(len(tok_lists)):
        tl = tok_lists[c][:cap]
        xt = np.zeros((H, cap), ml_dtypes.bfloat16)
        if len(tl):
            xt[:, :len(tl)] = x[tl].T
        m = {"xt": np.ascontiguousarray(
            xt.reshape(KH, P, cap).transpose(1, 0, 2))}
        wt = gup[c].T.astype(ml_dtypes.bfloat16).reshape(KH, P, 2 * I)
        for i, (m0, m1) in enumerate(GU_BLOCKS):
            m[f"wg{i}"] = np.ascontiguousarray(
                wt[:, :, m0 * P:m1 * P].transpose(1, 0, 2))
            m[f"wu{i}"] = np.ascontiguousarray(
                wt[:, :, I + m0 * P:I + m1 * P].transpose(1, 0, 2))
        dt_ = dp[c].T.astype(ml_dtypes.bfloat16).reshape(KI, P, H)
        for j in range(2):
            m[f"wd{j}"] = np.ascontiguousarray(
                dt_[:, :, j * NMAX:(j + 1) * NMAX].transpose(1, 0, 2))
        in_maps.append(m)
    return in_maps


def _gelu_tanh(g):
    return 0.5 * g * (1.0 + np.tanh(0.7978845608028654 * (g + 0.044715 * g * g * g)))


def _host_expert(x, gup_e, dp_e, toks):
    """Exact fp32 host-side gated MLP for overflow tokens."""
    gu = x[toks] @ gup_e.T
    gate, up = gu[:, :I], gu[:, I:]
    h = _gelu_tanh(gate) * up
    return h @ dp_e.T


def kernel(hidden_states, top_k_index, top_k_weights, gate_up_proj, down_proj):
    from concourse.bass_utils import run_bass_kernel_spmd

    x = np.asarray(hidden_states, dtype=np.float32)
    idx = np.asarray(top_k_index)
    tkw = np.asarray(top_k_weights, dtype=np.float32)
    gup = np.asarray(gate_up_proj, dtype=np.float32)
    dp = np.asarray(down_proj, dtype=np.float32)

    t, h = x.shape
    e = gup.shape[0]
    assert (t, h, e) == (T, H, E), (t, h, e)

    # Per-(token, expert) combine weights; duplicate top-k slots merge.
    ar = np.arange(t)
    combine = np.zeros((t, e), np.float32)
    np.add.at(combine, (ar[:, None], idx), tkw)
    pres = np.zeros((t, e), bool)
    pres[ar[:, None], idx] = True
    tok_lists = [np.nonzero(pres[:, c])[0] for c in range(e)]

    cap = _capacity(tok_lists)
    nc = _get_program(cap)
    in_maps = prepare_in_maps(x, gup, dp, tok_lists, cap)

    res = run_bass_kernel_spmd(nc, in_maps, list(range(e)))

    out = np.zeros((t, h), np.float32)
    for c in range(e):
        tl = tok_lists[c][:cap]
        if len(tl):
            yt = np.asarray(res.results[c]["yt"], np.float32).reshape(H, cap)
            out[tl] += combine[tl, c][:, None] * yt[:, :len(tl)].T
        spill = tok_lists[c][cap:]
        if len(spill):
            y = _host_expert(x, gup[c], dp[c], spill)
            out[spill] += combine[spill, c][:, None] * y
    return out

